# revision 9
# baseline (speedup 1.0000x reference)
"""Trainium2 Bass kernel: ViT-style global attention with decomposed
relative position bias (B=8, 32x32 tokens, dim 768, 12 heads, hd 64).

Sharding: data-parallel over batch B=8 -> one image per NeuronCore (8
cores), weights replicated, no collectives.

Per-core dataflow (all on-chip ops partition-preserving):
  1a  qkT = Wqk @ xT (feature-major), q pre-scaled by hd^-0.5 via host
      weight fold, per-partition bias add on DVE during PSUM->SBUF copy.
  1b  RELH/RELW (rel-pos tables contracted with q) via block-diagonal
      stationary matmuls; assembled with qT into Q' = [qT|RELH|RELW]
      (rows parity-mirrored for odd heads so every copy keeps partitions).
  1c  V computed token-major directly (xT as stationary), stored bf16
      with a ones column -> softmax denominators ride the PV matmul free.
  2   per (head, kblock): ONE K=128 matmul yields scale*S^T + rel_h +
      rel_w in PSUM (bias rides contraction rows 64..127 against constant
      indicator patterns in K'). exp on ScalarE -> P^T bf16. PV matmul
      (V''-stationary) -> (P@V)^T + denominator row. K=1 ones-matmul
      broadcasts the denominator row; reciprocal + multiply on DVE ->
      AOD feature-major.
  3   proj matmul, +proj_b on DVE, PE-transpose to token-major, DMA out.
"""

import numpy as np

import concourse.bacc as bacc
import concourse.tile as tile
from concourse import mybir
from concourse import bass_utils

B, H, W, DIM = 8, 32, 32, 768
HEADS, HD = 12, 64
N = H * W  # 1024
NCORES = 8
SCALE = HD ** -0.5
F32 = mybir.dt.float32
F32R = mybir.dt.float32r
BF16 = mybir.dt.bfloat16
EXP = mybir.ActivationFunctionType.Exp
ADD = mybir.AluOpType.add

NC = DIM // 128      # 6 feature chunks
NKB = N // 128       # 8 key blocks
NQH = N // 512       # 2 query halves
VW = 65 + 128        # even (V|1) + odd (0,1,0,V) stationary widths

_CACHE = {}


def build_nc():
    nc = bacc.Bacc("TRN2", target_bir_lowering=False, debug=False)

    xT = nc.dram_tensor("xT", (DIM, N), F32R, kind="ExternalInput").ap()
    wqkvT = nc.dram_tensor("wqkvT", (DIM, 3 * DIM), F32R, kind="ExternalInput").ap()
    qkvb = nc.dram_tensor("qkvb", (3 * DIM,), F32, kind="ExternalInput").ap()
    wprojT = nc.dram_tensor("wprojT", (DIM, DIM), F32R, kind="ExternalInput").ap()
    projb = nc.dram_tensor("projb", (DIM,), F32, kind="ExternalInput").ap()
    bdh = nc.dram_tensor("bdh", (H, 128, 128), F32R, kind="ExternalInput").ap()
    bdw = nc.dram_tensor("bdw", (W, 128, 128), F32R, kind="ExternalInput").ap()
    kconst = nc.dram_tensor("kconst", (64, N), F32R, kind="ExternalInput").ap()
    ident = nc.dram_tensor("ident", (128, 128), F32, kind="ExternalInput").ap()
    consd = nc.dram_tensor("consd", (128, 256), F32R, kind="ExternalInput").ap()
    vbrow = nc.dram_tensor("vbrow", (1, DIM), F32R, kind="ExternalInput").ap()
    y = nc.dram_tensor("y", (N, DIM), F32, kind="ExternalOutput").ap()

    qkvb2 = qkvb.rearrange("(c p one) -> c p one", p=128, one=1)   # [18][128,1]
    projb2 = projb.rearrange("(c p one) -> c p one", p=128, one=1)  # [6][128,1]
    vb_row = qkvb.rearrange("(one f) -> one f", one=1)              # [1,2304]

    with tile.TileContext(nc) as tc:
        # ---- long-lived pools (bottom of SBUF stack) ----
        qall_p = tc.alloc_tile_pool(name="qall", bufs=1)
        kall_p = tc.alloc_tile_pool(name="kall", bufs=1)
        vall_p = tc.alloc_tile_pool(name="vall", bufs=1)
        cons_p = tc.alloc_tile_pool(name="cons", bufs=1)

        QALL = qall_p.tile([128, HEADS, N], F32R)
        KALL = kall_p.tile([128, HEADS, N], F32R)
        VALL = vall_p.tile([128, HEADS // 2, NKB, VW], BF16)
        CONS = cons_p.tile([128, 256], F32R)
        IDENT = cons_p.tile([128, 128], F32)
        VBS = cons_p.tile([1, DIM], F32R)

        nc.sync.dma_start(out=IDENT, in_=ident)
        nc.sync.dma_start(out=VBS, in_=vbrow)
        nc.sync.dma_start(out=CONS, in_=consd)

        # V'' layout: even head cols [V(64)|1]; odd [0(32)|1|0(31)|V(64)]
        nc.vector.memset(VALL[:, :, :, 64:65], 1.0)
        nc.vector.memset(VALL[:, :, :, 65:97], 0.0)
        nc.vector.memset(VALL[:, :, :, 97:98], 1.0)
        nc.vector.memset(VALL[:, :, :, 98:129], 0.0)

        # constant bias-index patterns into K' (even rows 64:128, odd 0:64)
        for h in range(HEADS):
            rows = slice(64, 128) if h % 2 == 0 else slice(0, 64)
            nc.sync.dma_start(out=KALL[rows, h, :], in_=kconst)

        with tc.tile_pool(name="xtp", bufs=1) as xt_p:
            XT = xt_p.tile([128, NC, N], F32R)
            for c in range(NC):
                nc.sync.dma_start(out=XT[:, c, :], in_=xT[c * 128:(c + 1) * 128, :])

            with tc.tile_pool(name="stage", bufs=1) as stage_p:
                STQ = stage_p.tile([128, NC, N], F32R)

                # ---------- 1a: q & k features (feature-major) ----------
                with tc.tile_pool(name="wqk", bufs=2) as w_p, \
                     tc.tile_pool(name="bias", bufs=4) as b_p, \
                     tc.tile_pool(name="ps1", bufs=4, space="PSUM") as ps_p:
                    for g in range(4):           # 4 groups x 3 of-blocks
                        wt = w_p.tile([128, NC, 384], F32R, tag="wt")
                        for c in range(NC):
                            nc.sync.dma_start(
                                out=wt[:, c, :],
                                in_=wqkvT[c * 128:(c + 1) * 128,
                                          g * 384:(g + 1) * 384])
                        for mi in range(3):
                            m = g * 3 + mi       # 0..11 (q: 0-5, k: 6-11)
                            bias_t = b_p.tile([128, 1], F32, tag="bias")
                            nc.sync.dma_start(out=bias_t, in_=qkvb2[m])
                            for qh in range(NQH):
                                ps = ps_p.tile([128, 512], F32, tag="ps1")
                                for c in range(NC):
                                    nc.tensor.matmul(
                                        ps,
                                        lhsT=wt[:, c, mi * 128:(mi + 1) * 128],
                                        rhs=XT[:, c, qh * 512:(qh + 1) * 512],
                                        start=(c == 0), stop=(c == NC - 1))
                                qsl = slice(qh * 512, (qh + 1) * 512)
                                he, ho = 2 * (m % 6), 2 * (m % 6) + 1
                                if m < 6:
                                    nc.vector.tensor_scalar(
                                        out=STQ[:, m, qsl], in0=ps,
                                        scalar1=bias_t, scalar2=None, op0=ADD)
                                else:
                                    nc.vector.tensor_scalar(
                                        out=KALL[0:64, he, qsl], in0=ps[0:64],
                                        scalar1=bias_t[0:64], scalar2=None,
                                        op0=ADD)
                                    nc.vector.tensor_scalar(
                                        out=KALL[64:128, ho, qsl],
                                        in0=ps[64:128],
                                        scalar1=bias_t[64:128], scalar2=None,
                                        op0=ADD)

                # ---------- 1b: RELH/RELW + Q' assembly ----------
                for h in range(HEADS):
                    rows = slice(0, 64) if h % 2 == 0 else slice(64, 128)
                    nc.vector.tensor_copy(QALL[rows, h, :], STQ[rows, h // 2, :])

                q5 = QALL.rearrange("p (pr hh) n -> p pr hh n", hh=2)
                q6 = QALL.rearrange("p (pr hh) (t ww) -> p pr hh t ww",
                                    hh=2, ww=W)
                stq4 = STQ.rearrange("p c (t ww) -> p c t ww", ww=W)
                with tc.tile_pool(name="bd", bufs=4) as bd_p, \
                     tc.tile_pool(name="ps2", bufs=4, space="PSUM") as ps_p:
                    for h in range(H):
                        bdh_t = bd_p.tile([128, 128], F32R, tag="bd")
                        nc.sync.dma_start(out=bdh_t, in_=bdh[h])
                        ps_h = ps_p.tile([128, NC, 32], F32, tag="ps2")
                        nc.tensor.matmul(ps_h, lhsT=bdh_t,
                                         rhs=STQ[:, :, h * 32:(h + 1) * 32],
                                         start=True, stop=True)
                        nc.vector.tensor_copy(
                            q5[64:96, :, 0, h * 32:(h + 1) * 32], ps_h[64:96])
                        nc.vector.tensor_copy(
                            q5[0:32, :, 1, h * 32:(h + 1) * 32], ps_h[0:32])
                    for w in range(W):
                        bdw_t = bd_p.tile([128, 128], F32R, tag="bd")
                        nc.sync.dma_start(out=bdw_t, in_=bdw[w])
                        ps_w = ps_p.tile([128, NC, 32], F32, tag="ps2")
                        nc.tensor.matmul(ps_w, lhsT=bdw_t,
                                         rhs=stq4[:, :, :, w],
                                         start=True, stop=True)
                        nc.vector.tensor_copy(q6[96:128, :, 0, :, w],
                                              ps_w[96:128])
                        nc.vector.tensor_copy(q6[32:64, :, 1, :, w],
                                              ps_w[32:64])

            # ---------- 1c: V token-major, bf16, into V'' ----------
            with tc.tile_pool(name="wv", bufs=2) as w_p, \
                 tc.tile_pool(name="ps3", bufs=4, space="PSUM") as ps_p:
                for vh in range(2):              # two halves of 384 v-features
                    wt = w_p.tile([128, NC, 384], F32R, tag="wtv")
                    for c in range(NC):
                        nc.sync.dma_start(
                            out=wt[:, c, :],
                            in_=wqkvT[c * 128:(c + 1) * 128,
                                      2 * DIM + vh * 384:2 * DIM + (vh + 1) * 384])
                    for tb in range(NKB):        # 8 token blocks
                        ps = ps_p.tile([128, 384], F32, tag="ps3")
                        for c in range(NC):
                            nc.tensor.matmul(
                                ps, lhsT=XT[:, c, tb * 128:(tb + 1) * 128],
                                rhs=wt[:, c, :],
                                start=(c == 0), stop=False)
                        nc.tensor.matmul(
                            ps, lhsT=CONS[0:1, 0:128],
                            rhs=VBS[:, vh * 384:(vh + 1) * 384],
                            start=False, stop=True)
                        for j in range(6):
                            head = vh * 6 + j
                            pair, par = head // 2, head % 2
                            dst = (VALL[:, pair, tb, 0:64] if par == 0
                                   else VALL[:, pair, tb, 129:193])
                            nc.vector.tensor_copy(dst,
                                                  ps[:, j * 64:(j + 1) * 64])

        # ---------- 2: attention ----------
        aod_p = tc.alloc_tile_pool(name="aod", bufs=1)
        AOD = aod_p.tile([128, NC, N], F32R)
        with tc.tile_pool(name="pt", bufs=3) as pt_p, \
             tc.tile_pool(name="sm", bufs=4) as sm_p, \
             tc.tile_pool(name="pss", bufs=2, space="PSUM") as psS_p, \
             tc.tile_pool(name="pspv", bufs=2, space="PSUM") as psPV_p, \
             tc.tile_pool(name="psrb", bufs=2, space="PSUM") as psRB_p:
            for head in range(HEADS):
                pair, par = head // 2, head % 2
                pv = [psPV_p.tile([128, 512], F32, tag="pv", name=f"pv{head}_{qh}")
                      for qh in range(NQH)]
                vsl = (slice(0, 65) if par == 0 else slice(65, 193))
                for kb in range(NKB):
                    ps_s = psS_p.tile([128, 1024], F32, tag="pss")
                    for qh in range(NQH):
                        nc.tensor.matmul(
                            ps_s[:, qh * 512:(qh + 1) * 512],
                            lhsT=KALL[:, head, kb * 128:(kb + 1) * 128],
                            rhs=QALL[:, head, qh * 512:(qh + 1) * 512],
                            start=True, stop=True)
                    pt = pt_p.tile([128, 1024], BF16, tag="pt")
                    nc.scalar.activation(pt, ps_s, EXP)
                    for qh in range(NQH):
                        pv_out = pv[qh][0:65] if par == 0 else pv[qh]
                        nc.tensor.matmul(
                            pv_out, lhsT=VALL[:, pair, kb, vsl],
                            rhs=pt[:, qh * 512:(qh + 1) * 512],
                            start=(kb == 0), stop=(kb == NKB - 1))
                ao_rows = slice(0, 64) if par == 0 else slice(64, 128)
                dr = 64 if par == 0 else 32     # denominator row (32-aligned)
                for qh in range(NQH):
                    dsb = sm_p.tile([128, 512], F32R, tag="dsb")
                    nc.vector.tensor_copy(dsb[dr:dr + 1], pv[qh][dr:dr + 1])
                    rb = psRB_p.tile([128, 512], F32, tag="rb")
                    if par == 0:
                        nc.tensor.matmul(rb[0:64], lhsT=CONS[64:65, 0:64],
                                         rhs=dsb[64:65],
                                         start=True, stop=True)
                    else:
                        nc.tensor.matmul(rb,
                                         lhsT=CONS[32:33, 128:256],
                                         rhs=dsb[32:33],
                                         start=True, stop=True)
                    rbr = sm_p.tile([128, 512], F32, tag="rbr")
                    nc.vector.reciprocal(rbr[ao_rows], rb[ao_rows])
                    nc.vector.tensor_mul(
                        AOD[ao_rows, pair, qh * 512:(qh + 1) * 512],
                        pv[qh][ao_rows], rbr[ao_rows])

        # ---------- 3: proj + bias + transpose + out ----------
        with tc.tile_pool(name="ysb", bufs=1) as ysb_p, \
             tc.tile_pool(name="wp", bufs=2) as w_p, \
             tc.tile_pool(name="bias2", bufs=4) as b_p, \
             tc.tile_pool(name="ps4", bufs=4, space="PSUM") as ps_p, \
             tc.tile_pool(name="ps5", bufs=4, space="PSUM") as psT_p:
            YSB = ysb_p.tile([128, NC, N], F32)
            for g in range(2):
                wt = w_p.tile([128, NC, 384], F32R, tag="wtp")
                for c in range(NC):
                    nc.sync.dma_start(
                        out=wt[:, c, :],
                        in_=wprojT[c * 128:(c + 1) * 128,
                                   g * 384:(g + 1) * 384])
                for mi in range(3):
                    ob = g * 3 + mi
                    bias_t = b_p.tile([128, 1], F32, tag="bias2")
                    nc.sync.dma_start(out=bias_t, in_=projb2[ob])
                    for qh in range(NQH):
                        ps = ps_p.tile([128, 512], F32, tag="ps4")
                        for c in range(NC):
                            nc.tensor.matmul(
                                ps,
                                lhsT=wt[:, c, mi * 128:(mi + 1) * 128],
                                rhs=AOD[:, c, qh * 512:(qh + 1) * 512],
                                start=(c == 0), stop=(c == NC - 1))
                        nc.vector.tensor_scalar(
                            out=YSB[:, ob, qh * 512:(qh + 1) * 512], in0=ps,
                            scalar1=bias_t, scalar2=None, op0=ADD)
            for tb in range(NKB):
                tsl = slice(tb * 128, (tb + 1) * 128)
                for g, width in ((0, 512), (1, 256)):
                    pst = psT_p.tile([128, 512], F32, tag="ps5")
                    for j in range(width // 128):
                        ob = g * 4 + j
                        nc.tensor.matmul(
                            pst[:, j * 128:(j + 1) * 128],
                            lhsT=YSB[:, ob, tsl], rhs=IDENT,
                            is_transpose=True, start=True, stop=True,
                            skip_group_check=True)
                    yout = b_p.tile([128, 512], F32, tag="yout")
                    nc.vector.tensor_copy(yout[:, :width], pst[:, :width])
                    nc.sync.dma_start(
                        out=y[tsl, g * 512:g * 512 + width],
                        in_=yout[:, :width])
        aod_p.release()
        cons_p.release()
        vall_p.release()
        kall_p.release()
        qall_p.release()

    nc.compile()
    return nc


def host_prep(x, qkv_w, qkv_b, proj_w, proj_b, rel_pos_h, rel_pos_w):
    """full inputs -> list of 8 per-core in_maps"""
    x = np.asarray(x, np.float32)
    qkv_w = np.asarray(qkv_w, np.float32)
    qkv_b = np.asarray(qkv_b, np.float32)
    proj_w = np.asarray(proj_w, np.float32)
    proj_b = np.asarray(proj_b, np.float32)
    rel_pos_h = np.asarray(rel_pos_h, np.float32)
    rel_pos_w = np.asarray(rel_pos_w, np.float32)

    wqkvT = np.ascontiguousarray(qkv_w.T).copy()
    wqkvT[:, :DIM] *= SCALE
    qkvb2 = qkv_b.copy()
    qkvb2[:DIM] *= SCALE
    wprojT = np.ascontiguousarray(proj_w.T)

    idx = np.arange(H)
    Rh = rel_pos_h[idx[:, None] - idx[None, :] + (H - 1)]  # (32,32,64)
    Rw = rel_pos_w[idx[:, None] - idx[None, :] + (W - 1)]
    bdh = np.zeros((H, 128, 128), np.float32)
    bdw = np.zeros((W, 128, 128), np.float32)
    for h in range(H):
        bdh[h, 0:64, 64:96] = Rh[h].T / SCALE
        bdh[h, 64:128, 0:32] = Rh[h].T / SCALE
    for w in range(W):
        bdw[w, 0:64, 96:128] = Rw[w].T / SCALE
        bdw[w, 64:128, 32:64] = Rw[w].T / SCALE

    k = np.arange(N)
    kconst = np.zeros((64, N), np.float32)
    kconst[:32] = (k[None, :] // 32 == np.arange(32)[:, None])
    kconst[32:] = (k[None, :] % 32 == np.arange(32)[:, None])

    ident = np.eye(128, dtype=np.float32)
    consd = np.zeros((128, 256), np.float32)
    consd[:, 0:128] = 1.0
    consd[:, 192:256] = 1.0
    vbrow = np.ascontiguousarray(qkvb2[2 * DIM:].reshape(1, DIM))

    shared = dict(wqkvT=wqkvT, qkvb=qkvb2, wprojT=wprojT, projb=proj_b,
                  bdh=bdh, bdw=bdw, kconst=kconst, ident=ident,
                  consd=consd, vbrow=vbrow)
    in_maps = []
    for b in range(B):
        xT = np.ascontiguousarray(x[b].reshape(N, DIM).T)
        in_maps.append(dict(xT=xT, **shared))
    return in_maps


def get_nc():
    if "nc" not in _CACHE:
        _CACHE["nc"] = build_nc()
    return _CACHE["nc"]


def kernel(**inputs):
    nc = get_nc()
    in_maps = host_prep(**inputs)
    res = bass_utils.run_bass_kernel_spmd(nc, in_maps, core_ids=list(range(NCORES)))
    out = np.stack([np.asarray(r["y"]) for r in res.results], axis=0)
    return out.reshape(B, H, W, DIM).astype(np.float32)
